# revision 13
# baseline (speedup 1.0000x reference)
"""Trainium2 Bass kernel for nn_EnhancementGenerator.

Math: the reference is a (buggy, non-recurrent) bidirectional 2-layer GRU
applied pointwise over (B,T), followed by an efficient-kan KANLinear and
1.2*sigmoid(slope*out).  Everything is row-pointwise except that the
backward direction pairs output row (b,t) with input row (b,T-1-t).

Reformulation (validated against the jax reference):
  * GRU: no recurrence => 4 independent "cells".  Layer-0 sees h=0.  Both
    directions are packed into [f(40); b(40)] partition tiles (b at 64);
    the time reversal is applied once at feat-assembly with a reversed
    free-dim access pattern.
  * z-gates are computed as z' = sigmoid(-x) = 1-z (activation scale=-1,
    negated bias) so all gate combines are plain tensor_tensor ops that hit
    the DVE fp16 2x mode; the r*bhh_n term of layer 0 rides a diagonal
    matmul into the n-psum so no DVE op touches it at all.
  * KAN spline: uniform-knot B-splines == truncated cubic powers.  feat
    lies in (-1,1) so only knots {-0.6,-0.2,0.2,0.6} produce kinks; the
    rest fold into a cubic polynomial.  Knot basis = clip(feat-t,0)^3 via
    one 4x tensor_scalar + two 2x tensor_tensor muls.
  * KAN matmuls are row-major-out: out[rows,257] = powers^T @ wkan with the
    powers stationary (K=128 incl. zero pad) and all 257 output features in
    the moving free dim.  The constant term rides a persistent ones-row at
    partition 104 of the feat tiles (wkan row 104 of the feat slot = a0).
  * x feature 256 (the 257th) is handled by K=1 matmuls placed in distinct
    32-row PE tile positions, so the three tail matmuls of a GRU stage run
    concurrently.
  * Output is written row-major bf16 in [128, 8, 257] chunk layout (two
    DMAs per sample); the final *1.2 happens on host.
"""
import os
import sys

for _p in (
    "/root/.axon_site",
    "/root/.axon_site/_ro/trn_rl_repo",
    "/root/.axon_site/_ro/pypackages",
    "/opt/trn_rl_repo",
    "/opt/pypackages",
):
    if os.path.isdir(_p) and _p not in sys.path:
        sys.path.append(_p)

import numpy as np

import concourse.bass as bass
import concourse.tile as tile
from concourse import bacc, mybir
from concourse.bass_utils import run_bass_kernel_spmd

F32 = mybir.dt.float32
BF16 = mybir.dt.bfloat16
FP16 = mybir.dt.float16
AF = mybir.ActivationFunctionType
ALU = mybir.AluOpType

N_CORES = 8
B, T, IN_SIZE, HID, OUT_SIZE = 64, 1000, 257, 40, 257
NT = 500            # rows per half-sample
NT2 = 2 * NT
SPB = B // N_CORES  # samples per core
ROWS = SPB * T      # rows per core
KNOTS = [(-0.6, "L"), (-0.2, "L"), (0.2, "R"), (0.6, "R")]
PG = 104            # packed direction block: f at 0:40, b at 64:104
BO = 64             # b-direction partition offset


# --------------------------------------------------------------------------
# host-side weight folding
# --------------------------------------------------------------------------
def fold_weights(inp):
    from math import comb
    W = {k: np.asarray(v, dtype=np.float64) for k, v in inp.items()}
    out = {}
    # gi weights, plane-major: wgi[k, ci, (l*3+g)*PG + {f:0:40, b:64:104}]
    wgi = np.zeros((128, 2, 6 * PG))
    wtail = np.zeros((128, 6 * PG))
    for l in range(2):
        for g in range(3):
            c0 = (l * 3 + g) * PG
            bf = W["Wih_f"][l][g * 40:(g + 1) * 40]   # (40, 257)
            bb = W["Wih_b"][l][g * 40:(g + 1) * 40]
            for ci in range(2):
                wgi[:, ci, c0:c0 + 40] = bf[:, ci * 128:(ci + 1) * 128].T
                wgi[:, ci, c0 + BO:c0 + BO + 40] = bb[:, ci * 128:(ci + 1) * 128].T
            for q in range(4):
                wtail[32 * q, c0:c0 + 40] = bf[:, 256]
                wtail[32 * q, c0 + BO:c0 + BO + 40] = bb[:, 256]
    out["wgi"] = wgi
    out["wtail"] = wtail
    # gh (positive now, blockdiag): (PG, 3*PG)
    wgh = np.zeros((PG, 3 * PG))
    for g in range(3):
        wgh[0:40, g * PG:g * PG + 40] = W["Whh_f"][1][g * 40:(g + 1) * 40].T
        wgh[BO:BO + 40, g * PG + BO:g * PG + BO + 40] = W["Whh_b"][1][g * 40:(g + 1) * 40].T
    out["wgh"] = wgh
    # diag(bhh_n layer0): r1*bhh_n1 rides a matmul into the n-psum
    wdiag = np.zeros((PG, PG))
    wdiag[np.arange(40), np.arange(40)] = W["bhh_f"][0][80:120]
    wdiag[np.arange(BO, BO + 40), np.arange(BO, BO + 40)] = W["bhh_b"][0][80:120]
    out["wdiag"] = wdiag
    # gru biases: (PG, 8); z-gate biases negated (z' = sigmoid(-x - b))
    bg = np.zeros((PG, 8))
    for l in range(2):
        for gi_ in range(2):
            sgn = -1.0 if gi_ == 1 else 1.0
            bg[0:40, l * 4 + gi_] = sgn * (W["bih_f"][l][gi_ * 40:(gi_ + 1) * 40]
                                           + W["bhh_f"][l][gi_ * 40:(gi_ + 1) * 40])
            bg[BO:BO + 40, l * 4 + gi_] = sgn * (W["bih_b"][l][gi_ * 40:(gi_ + 1) * 40]
                                                 + W["bhh_b"][l][gi_ * 40:(gi_ + 1) * 40])
        bg[0:40, l * 4 + 2] = W["bhh_f"][l][80:120]
        bg[BO:BO + 40, l * 4 + 2] = W["bhh_b"][l][80:120]
        bg[0:40, l * 4 + 3] = W["bih_f"][l][80:120]
        bg[BO:BO + 40, l * 4 + 3] = W["bih_b"][l][80:120]
    out["bgru"] = bg
    # KAN: truncated-power reformulation
    h = 0.4
    t = -2.2 + h * np.arange(12)
    w = W["spline_weight"] * W["spline_scaler"][..., None]          # (257, 80, 8)
    s = np.zeros((8, 12))
    for m in range(8):
        for k in range(5):
            s[m, m + k] = ((-1) ** k) * comb(4, k) / (6 * h ** 3)
    V = np.einsum("oim,mj->oij", w, s)                              # (257, 80, 12)
    A = np.zeros((4, 257, 80))
    for j in range(6):
        for d in range(4):
            A[d] += V[:, :, j] * comb(3, d) * ((-t[j]) ** (3 - d))
    slope = W["slope"]
    # wkan row-major rhs: (128, 8, 257); row 104 of the feat slot = a0.
    wkan = np.zeros((128, 8, 257))
    mats = [W["base_weight"].T, A[1].T, A[2].T, A[3].T] + [
        -V[:, :, 4].T, -V[:, :, 5].T, V[:, :, 6].T, V[:, :, 7].T]
    for idx, m in enumerate(mats):  # m: (80, 257)
        ms = m * slope[None, :]
        wkan[0:40, idx] = ms[0:40]
        wkan[BO:BO + 40, idx] = ms[40:80]
    wkan[104, 1] = A[0].sum(axis=1) * slope
    out["wkan"] = wkan
    out["ones1"] = np.ones((1, NT2))
    return {k: np.ascontiguousarray(v, dtype=np.float32) for k, v in out.items()}


# --------------------------------------------------------------------------
# device kernel
# --------------------------------------------------------------------------
def build_nc(n_samples=SPB):
    rows = n_samples * T
    nc = bacc.Bacc("TRN2", target_bir_lowering=False, debug=False)

    def mm(out, lhsT, rhs, **kw):
        nc.tensor.matmul(out, lhsT, rhs, **kw)

    xt_d = nc.dram_tensor("xt", [IN_SIZE, rows], FP16, kind="ExternalInput")
    wgi_d = nc.dram_tensor("wgi", [128, 2, 6 * PG], FP16, kind="ExternalInput")
    wtail_d = nc.dram_tensor("wtail", [128, 6 * PG], FP16, kind="ExternalInput")
    wgh_d = nc.dram_tensor("wgh", [PG, 3 * PG], FP16, kind="ExternalInput")
    wdiag_d = nc.dram_tensor("wdiag", [PG, PG], FP16, kind="ExternalInput")
    wkan_d = nc.dram_tensor("wkan", [128, 8, 257], FP16, kind="ExternalInput")
    bgru_d = nc.dram_tensor("bgru", [PG, 8], F32, kind="ExternalInput")
    ones_d = nc.dram_tensor("ones1", [1, NT2], FP16, kind="ExternalInput")
    yt_d = nc.dram_tensor("yt", [n_samples, 128, 8, 257], BF16, kind="ExternalOutput")

    with tile.TileContext(nc) as tc:
        with (
            tc.tile_pool(name="wts", bufs=1) as wp,
            tc.tile_pool(name="xin", bufs=3) as xp,
            tc.tile_pool(name="work", bufs=1) as kp,
            tc.tile_pool(name="outp", bufs=2) as op_,
            tc.tile_pool(name="psg", bufs=1, space="PSUM") as psg,
            tc.tile_pool(name="psk", bufs=1, space="PSUM") as psk,
        ):
            # ---- resident weights
            wgi_sb = wp.tile([128, 2, 6 * PG], FP16, tag="wgi")
            nc.sync.dma_start(wgi_sb[:], wgi_d[:])
            wtail_sb = wp.tile([128, 6 * PG], FP16, tag="wtail")
            nc.sync.dma_start(wtail_sb[:], wtail_d[:])
            wgh_sb = wp.tile([PG, 3 * PG], FP16, tag="wgh")
            nc.sync.dma_start(wgh_sb[:], wgh_d[:])
            wdiag_sb = wp.tile([PG, PG], FP16, tag="wdiag")
            nc.sync.dma_start(wdiag_sb[:], wdiag_d[:])
            wkan_sb = wp.tile([128, 8, 257], FP16, tag="wkan")
            nc.sync.dma_start(wkan_sb[:], wkan_d[:])
            bg = wp.tile([PG, 8], F32, tag="bgru")
            nc.sync.dma_start(bg[:], bgru_d[:])

            # persistent feat tiles: gap 40:64 and pad 105:128 stay zero,
            # partition 104 stays 1.0 (the KAN constant-term row).
            featb = []
            for i in range(2):
                f = wp.tile([128, NT2], FP16, tag=f"featP{i}", name=f"featP{i}")
                nc.vector.memset(f[32:64, :], 0.0)
                nc.vector.memset(f[96:128, :], 0.0)
                nc.sync.dma_start(f[104:105, :], ones_d[:])
                featb.append(f)

            S = [dict() for _ in range(n_samples)]

            def load_x(smp):
                st = S[smp]
                s0 = smp * T
                xm = xp.tile([128, 2, NT2], FP16, tag="xm", name="xm")
                for ci in range(2):
                    nc.sync.dma_start(xm[:, ci, :],
                                      xt_d[ci * 128:(ci + 1) * 128, s0:s0 + NT2])
                xt4 = xp.tile([128, NT2], FP16, tag="xt4", name="xt4")
                for q in range(4):
                    nc.sync.dma_start(xt4[32 * q:32 * q + 1, :],
                                      xt_d[256:257, s0:s0 + NT2])
                st["xm"], st["xt4"] = xm, xt4

            def gi_main(ps, xm, lyr, g, hs):
                c0 = (lyr * 3 + g) * PG
                for ci in range(2):
                    mm(ps[:], wgi_sb[:, ci, c0:c0 + PG], xm[:, ci, hs],
                       start=(ci == 0), stop=False)

            def gi_tail(ps, xt4, lyr, g, hs, q, stop=True):
                c0 = (lyr * 3 + g) * PG
                mm(ps[:], wtail_sb[32 * q:32 * q + 1, c0:c0 + PG],
                   xt4[32 * q:32 * q + 1, hs], start=False, stop=stop,
                   tile_position=(32 * q, 0))

            def emit_l0(smp):
                st = S[smp]
                xm, xt4 = st["xm"], st["xt4"]
                rt = kp.tile([PG, NT2], FP16, tag="rt", bufs=2)
                zt = kp.tile([PG, NT2], FP16, tag="zt", bufs=2)
                n1 = kp.tile([PG, NT2], FP16, tag="n1", bufs=2)
                h1 = kp.tile([PG, NT2], FP16, tag="h1", bufs=2)
                for h in range(2):
                    hs = slice(h * NT, (h + 1) * NT)
                    ps_r = psg.tile([PG, NT], F32, tag="G", name="ps_r", bufs=6)
                    gi_main(ps_r, xm, 0, 0, hs)
                    gi_tail(ps_r, xt4, 0, 0, hs, 0)
                    ps_z = psg.tile([PG, NT], F32, tag="G", name="ps_z", bufs=6)
                    gi_main(ps_z, xm, 0, 1, hs)
                    ps_n = psg.tile([PG, NT], F32, tag="G", name="ps_n", bufs=6)
                    gi_main(ps_n, xm, 0, 2, hs)
                    gi_tail(ps_z, xt4, 0, 1, hs, 1)
                    gi_tail(ps_n, xt4, 0, 2, hs, 2, stop=False)
                    nc.scalar.activation(rt[:, hs], ps_r[:], AF.Sigmoid, bias=bg[:, 0:1])
                    nc.scalar.activation(zt[:, hs], ps_z[:], AF.Sigmoid,
                                         bias=bg[:, 1:2], scale=-1.0)
                    # n-psum += diag(bhh_n1) @ r  (replaces a DVE stt)
                    mm(ps_n[:], wdiag_sb[:], rt[:, hs], start=False, stop=True)
                    nc.scalar.activation(n1[:, hs], ps_n[:], AF.Tanh, bias=bg[:, 3:4])
                    nc.vector.tensor_mul(h1[:, hs], zt[:, hs], n1[:, hs])  # (1-z1)*n1
                st["h1"] = h1

            def emit_l1(smp):
                st = S[smp]
                xm, xt4, h1 = st["xm"], st["xt4"], st["h1"]
                r2t = kp.tile([PG, NT2], FP16, tag="r2t", bufs=2)
                z2t = kp.tile([PG, NT2], FP16, tag="z2t", bufs=2)
                t2t = kp.tile([PG, NT2], FP16, tag="t2t", bufs=2)
                vt = kp.tile([PG, NT2], FP16, tag="vt", bufs=2)
                for h in range(2):
                    hs = slice(h * NT, (h + 1) * NT)
                    ps_r2 = psg.tile([PG, NT], F32, tag="G", name="ps_r2", bufs=6)
                    gi_main(ps_r2, xm, 1, 0, hs)
                    mm(ps_r2[:], wgh_sb[:, 0:PG], h1[:, hs], start=False, stop=False)
                    gi_tail(ps_r2, xt4, 1, 0, hs, 0)
                    ps_z2 = psg.tile([PG, NT], F32, tag="G", name="ps_z2", bufs=6)
                    gi_main(ps_z2, xm, 1, 1, hs)
                    mm(ps_z2[:], wgh_sb[:, PG:2 * PG], h1[:, hs], start=False, stop=False)
                    ps_n2 = psg.tile([PG, NT], F32, tag="G", name="ps_n2", bufs=6)
                    gi_main(ps_n2, xm, 1, 2, hs)
                    ps_p3 = psg.tile([PG, NT], F32, tag="G", name="ps_p3", bufs=6)
                    mm(ps_p3[:], wgh_sb[:, 2 * PG:3 * PG], h1[:, hs], start=True, stop=True)
                    gi_tail(ps_z2, xt4, 1, 1, hs, 1)
                    gi_tail(ps_n2, xt4, 1, 2, hs, 2)
                    nc.scalar.activation(r2t[:, hs], ps_r2[:], AF.Sigmoid, bias=bg[:, 4:5])
                    nc.scalar.activation(z2t[:, hs], ps_z2[:], AF.Sigmoid,
                                         bias=bg[:, 5:6], scale=-1.0)
                    nc.vector.scalar_tensor_tensor(
                        t2t[:, hs], ps_p3[:], bg[:, 6:7], r2t[:, hs],
                        op0=ALU.add, op1=ALU.mult)
                    nc.vector.tensor_add(vt[:, hs], t2t[:, hs], ps_n2[:])
                n2 = kp.tile([PG, NT2], FP16, tag="n2", bufs=2)
                nc.scalar.activation(n2[:], vt[:], AF.Tanh, bias=bg[:, 7:8])
                # feat = (1-z2)*n2 + z2*h1 = z2t'*(n2-h1) + h1   (z2t' = 1-z2)
                wdt = kp.tile([PG, NT2], FP16, tag="wdt", bufs=2)
                nc.vector.tensor_sub(wdt[:], n2[:], h1[:])
                ut = kp.tile([PG, NT2], FP16, tag="ut", bufs=2)
                nc.vector.tensor_mul(ut[:], z2t[:], wdt[:])
                feat = featb[smp % 2]
                nc.vector.tensor_add(feat[0:40, :], ut[0:40, :], h1[0:40, :])
                nc.vector.tensor_add(feat[BO:BO + 40, :], ut[BO:BO + 40, ::-1],
                                     h1[BO:BO + 40, ::-1])
                st["feat"] = feat
                # KAN power basis (all fp16); breadth-first so no DVE op waits
                # on the instruction right before it.
                s2 = kp.tile([128, NT2], FP16, tag="s2", bufs=2)
                nc.gpsimd.tensor_mul(s2[:], feat[:], feat[:])
                sg = kp.tile([128, NT2], FP16, tag="sg", bufs=2)
                nc.scalar.activation(sg[:], feat[:], AF.Sigmoid)
                rjs = []
                for ji, (tj, side) in enumerate(KNOTS):
                    rj = kp.tile([128, NT2], FP16, tag=f"rj{ji}", name=f"rj{ji}", bufs=2)
                    nc.vector.tensor_scalar(
                        rj[:], feat[:], float(tj), 0.0, op0=ALU.subtract,
                        op1=(ALU.min if side == "L" else ALU.max))
                    rjs.append(rj)
                rjc = kp.tile([128, NT2], FP16, tag="rjc", bufs=2)
                nc.vector.tensor_copy(rjc[:], rjs[1][:])
                rjc3 = kp.tile([128, NT2], FP16, tag="rjc3", bufs=2)
                nc.vector.tensor_copy(rjc3[:], rjs[3][:])
                sl = kp.tile([128, NT2], FP16, tag="sl", bufs=2)
                nc.gpsimd.tensor_mul(sl[:], sg[:], feat[:])
                s3 = kp.tile([128, NT2], FP16, tag="s3", bufs=2)
                nc.vector.tensor_mul(s3[:], s2[:], feat[:])
                qjs = []
                for ji in range(4):
                    qj = kp.tile([128, NT2], FP16, tag=f"qj{ji}", name=f"qj{ji}", bufs=2)
                    if ji % 2 == 0:
                        nc.gpsimd.tensor_mul(qj[:], rjs[ji][:], rjs[ji][:])
                    else:
                        nc.vector.tensor_mul(qj[:], rjs[ji][:],
                                             (rjc if ji == 1 else rjc3)[:])
                    qjs.append(qj)
                powers = [sl, feat, s2, s3]
                for ji in range(4):
                    pj = kp.tile([128, NT2], FP16, tag=f"pj{ji}", name=f"pj{ji}", bufs=2)
                    nc.vector.tensor_mul(pj[:], qjs[ji][:], rjs[ji][:])
                    powers.append(pj)
                st["powers"] = powers

            def emit_kan(smp):
                st = S[smp]
                powers = st["powers"]
                ob = op_.tile([128, 8, 257], BF16, tag="ob", name="ob")
                for c in range(8):
                    r0 = c * 128
                    msz = min(128, T - r0)
                    rs = slice(r0, r0 + msz)
                    po = psk.tile([128, 257], F32, tag="K", name="po", bufs=2)
                    for idx, p in enumerate(powers):
                        mm(po[0:msz, :], p[:, rs], wkan_sb[:, idx, :],
                           start=(idx == 0), stop=(idx == 7))
                    nc.scalar.activation(ob[0:msz, c, :], po[0:msz, :], AF.Sigmoid)
                nc.sync.dma_start(yt_d[smp, :, 0:7, :], ob[:, 0:7, :])
                nc.sync.dma_start(yt_d[smp, 0:104, 7, :], ob[0:104, 7, :])

            load_x(0)
            if n_samples > 1:
                load_x(1)
            for k in range(n_samples + 2):
                if 0 <= k - 2 < n_samples:
                    emit_kan(k - 2)
                if k + 2 < n_samples:
                    load_x(k + 2)
                if k < n_samples:
                    emit_l0(k)
                if 0 <= k - 1 < n_samples:
                    emit_l1(k - 1)
                if 0 <= k - 2 < n_samples:
                    S[k - 2].clear()
    nc.compile()
    return nc


# --------------------------------------------------------------------------
# host entry point
# --------------------------------------------------------------------------
_NC_CACHE = {}


def _get_nc(n_samples=SPB, mode=None):
    key = n_samples
    if key not in _NC_CACHE:
        _NC_CACHE[key] = build_nc(n_samples)
    return _NC_CACHE[key]


def make_in_maps(inputs, n_samples=SPB, n_cores=N_CORES, mode=None):
    x = np.asarray(inputs["x"], dtype=np.float32)
    Wf = fold_weights(inputs)
    for k in ("wgi", "wtail", "wgh", "wdiag", "wkan", "ones1"):
        Wf[k] = np.ascontiguousarray(Wf[k].astype(np.float16))
    in_maps = []
    for c in range(n_cores):
        xc = x[c * n_samples:(c + 1) * n_samples].reshape(n_samples * T, IN_SIZE)
        xt = np.ascontiguousarray(xc.T.astype(np.float16))
        in_maps.append({"xt": xt, **Wf})
    return in_maps


def kernel(**inputs):
    x = np.asarray(inputs["x"], dtype=np.float32)
    assert x.shape == (B, T, IN_SIZE), x.shape
    nc = _get_nc(SPB)
    in_maps = make_in_maps(inputs)
    res = run_bass_kernel_spmd(nc, in_maps, list(range(N_CORES)))
    out = np.empty((B, T, OUT_SIZE), dtype=np.float32)
    for c in range(N_CORES):
        yt = np.asarray(res.results[c]["yt"], dtype=np.float32)  # (SPB,128,8,257)
        for s in range(SPB):
            y = yt[s].transpose(1, 0, 2).reshape(1024, OUT_SIZE)[:T]
            out[c * SPB + s] = 1.2 * y
    return out


if __name__ == "__main__":
    rng = np.random.default_rng(0)
    demo = {
        "x": rng.standard_normal((B, T, IN_SIZE), dtype=np.float32),
        "Wih_f": rng.standard_normal((2, 120, 257), dtype=np.float32) * 0.1,
        "Whh_f": rng.standard_normal((2, 120, 40), dtype=np.float32) * 0.1,
        "bih_f": rng.standard_normal((2, 120), dtype=np.float32) * 0.1,
        "bhh_f": rng.standard_normal((2, 120), dtype=np.float32) * 0.1,
        "Wih_b": rng.standard_normal((2, 120, 257), dtype=np.float32) * 0.1,
        "Whh_b": rng.standard_normal((2, 120, 40), dtype=np.float32) * 0.1,
        "bih_b": rng.standard_normal((2, 120), dtype=np.float32) * 0.1,
        "bhh_b": rng.standard_normal((2, 120), dtype=np.float32) * 0.1,
        "base_weight": rng.standard_normal((257, 80), dtype=np.float32) * 0.1,
        "spline_weight": rng.standard_normal((257, 80, 8), dtype=np.float32) * 0.1,
        "spline_scaler": np.ones((257, 80), dtype=np.float32),
        "slope": np.ones((257,), dtype=np.float32),
        "lengths": np.full((64,), 1000, dtype=np.int32),
    }
    out = kernel(**demo)
    print("kernel ran, out:", out.shape, out.dtype, float(out.min()), float(out.max()))


# revision 15
# speedup vs baseline: 1.1200x; 1.1200x over previous
"""Trainium2 Bass kernel for nn_EnhancementGenerator.

Math: the reference is a (buggy, non-recurrent) bidirectional 2-layer GRU
applied pointwise over (B,T), followed by an efficient-kan KANLinear and
1.2*sigmoid(slope*out).  Everything is row-pointwise except that the
backward direction pairs output row (b,t) with input row (b,T-1-t).

Reformulation (validated against the jax reference):
  * GRU: no recurrence => 4 independent "cells".  Layer-0 sees h=0.  Both
    directions are packed into [f(40); b(40)] partition tiles (b at 64);
    the time reversal is applied once at feat-assembly with a reversed
    free-dim access pattern.
  * z-gates are computed as z' = sigmoid(-x) = 1-z (activation scale=-1,
    negated bias) so all gate combines are plain tensor_tensor ops that hit
    the DVE fp16 2x mode; the r*bhh_n term of layer 0 rides a diagonal
    matmul into the n-psum so no DVE op touches it at all.
  * KAN spline: uniform-knot B-splines == truncated cubic powers.  feat
    lies in (-1,1) so only knots {-0.6,-0.2,0.2,0.6} produce kinks; the
    rest fold into a cubic polynomial.  Knot basis = clip(feat-t,0)^3 via
    one 4x tensor_scalar + two 2x tensor_tensor muls.
  * KAN matmuls are row-major-out: out[rows,257] = powers^T @ wkan with the
    powers stationary (K=128 incl. zero pad) and all 257 output features in
    the moving free dim.  The constant term rides a persistent ones-row at
    partition 104 of the feat tiles (wkan row 104 of the feat slot = a0).
  * x feature 256 (the 257th) is handled by K=1 matmuls placed in distinct
    32-row PE tile positions, so the three tail matmuls of a GRU stage run
    concurrently.
  * Output is written row-major bf16 in [128, 8, 257] chunk layout (two
    DMAs per sample); the final *1.2 happens on host.
"""
import os
import sys

for _p in (
    "/root/.axon_site",
    "/root/.axon_site/_ro/trn_rl_repo",
    "/root/.axon_site/_ro/pypackages",
    "/opt/trn_rl_repo",
    "/opt/pypackages",
):
    if os.path.isdir(_p) and _p not in sys.path:
        sys.path.append(_p)

import numpy as np

import concourse.bass as bass
import concourse.tile as tile
from concourse import bacc, mybir
from concourse.bass_utils import run_bass_kernel_spmd

F32 = mybir.dt.float32
BF16 = mybir.dt.bfloat16
FP16 = mybir.dt.float16
AF = mybir.ActivationFunctionType
ALU = mybir.AluOpType

N_CORES = 8
B, T, IN_SIZE, HID, OUT_SIZE = 64, 1000, 257, 40, 257
NT = 500            # rows per half-sample
NT2 = 2 * NT
SPB = B // N_CORES  # samples per core
ROWS = SPB * T      # rows per core
KNOTS = [(-0.6, "L"), (-0.2, "L"), (0.2, "R"), (0.6, "R")]
PG = 104            # packed direction block: f at 0:40, b at 64:104
BO = 64             # b-direction partition offset


# --------------------------------------------------------------------------
# host-side weight folding
# --------------------------------------------------------------------------
def fold_weights(inp):
    from math import comb
    W = {k: np.asarray(v, dtype=np.float64) for k, v in inp.items()}
    out = {}
    # gi weights, plane-major: wgi[k, ci, (l*3+g)*PG + {f:0:40, b:64:104}]
    wgi = np.zeros((128, 2, 6 * PG))
    wtail = np.zeros((128, 6 * PG))
    for l in range(2):
        for g in range(3):
            c0 = (l * 3 + g) * PG
            bf = W["Wih_f"][l][g * 40:(g + 1) * 40]   # (40, 257)
            bb = W["Wih_b"][l][g * 40:(g + 1) * 40]
            for ci in range(2):
                wgi[:, ci, c0:c0 + 40] = bf[:, ci * 128:(ci + 1) * 128].T
                wgi[:, ci, c0 + BO:c0 + BO + 40] = bb[:, ci * 128:(ci + 1) * 128].T
            for q in range(4):
                wtail[32 * q, c0:c0 + 40] = bf[:, 256]
                wtail[32 * q, c0 + BO:c0 + BO + 40] = bb[:, 256]
    out["wgi"] = wgi
    out["wtail"] = wtail
    # gh (positive now, blockdiag): (PG, 3*PG)
    wgh = np.zeros((PG, 3 * PG))
    for g in range(3):
        wgh[0:40, g * PG:g * PG + 40] = W["Whh_f"][1][g * 40:(g + 1) * 40].T
        wgh[BO:BO + 40, g * PG + BO:g * PG + BO + 40] = W["Whh_b"][1][g * 40:(g + 1) * 40].T
    out["wgh"] = wgh
    # diag(bhh_n layer0): r1*bhh_n1 rides a matmul into the n-psum
    wdiag = np.zeros((PG, PG))
    wdiag[np.arange(40), np.arange(40)] = W["bhh_f"][0][80:120]
    wdiag[np.arange(BO, BO + 40), np.arange(BO, BO + 40)] = W["bhh_b"][0][80:120]
    out["wdiag"] = wdiag
    # gru biases: (PG, 8); z-gate biases negated (z' = sigmoid(-x - b))
    bg = np.zeros((PG, 8))
    for l in range(2):
        for gi_ in range(2):
            sgn = -1.0 if gi_ == 1 else 1.0
            bg[0:40, l * 4 + gi_] = sgn * (W["bih_f"][l][gi_ * 40:(gi_ + 1) * 40]
                                           + W["bhh_f"][l][gi_ * 40:(gi_ + 1) * 40])
            bg[BO:BO + 40, l * 4 + gi_] = sgn * (W["bih_b"][l][gi_ * 40:(gi_ + 1) * 40]
                                                 + W["bhh_b"][l][gi_ * 40:(gi_ + 1) * 40])
        bg[0:40, l * 4 + 2] = W["bhh_f"][l][80:120]
        bg[BO:BO + 40, l * 4 + 2] = W["bhh_b"][l][80:120]
        bg[0:40, l * 4 + 3] = W["bih_f"][l][80:120]
        bg[BO:BO + 40, l * 4 + 3] = W["bih_b"][l][80:120]
    out["bgru"] = bg
    # KAN: truncated-power reformulation
    h = 0.4
    t = -2.2 + h * np.arange(12)
    w = W["spline_weight"] * W["spline_scaler"][..., None]          # (257, 80, 8)
    s = np.zeros((8, 12))
    for m in range(8):
        for k in range(5):
            s[m, m + k] = ((-1) ** k) * comb(4, k) / (6 * h ** 3)
    V = np.einsum("oim,mj->oij", w, s)                              # (257, 80, 12)
    A = np.zeros((4, 257, 80))
    for j in range(6):
        for d in range(4):
            A[d] += V[:, :, j] * comb(3, d) * ((-t[j]) ** (3 - d))
    slope = W["slope"]
    # wkan row-major rhs: (128, 8, 257); row 104 of the feat slot = a0.
    wkan = np.zeros((128, 8, 257))
    mats = [W["base_weight"].T, A[1].T, A[2].T, A[3].T] + [
        -V[:, :, 4].T, -V[:, :, 5].T, V[:, :, 6].T, V[:, :, 7].T]
    for idx, m in enumerate(mats):  # m: (80, 257)
        ms = m * slope[None, :]
        wkan[0:40, idx] = ms[0:40]
        wkan[BO:BO + 40, idx] = ms[40:80]
    wkan[104, 1] = A[0].sum(axis=1) * slope
    out["wkan"] = wkan
    out["ones1"] = np.ones((1, NT2))
    return {k: np.ascontiguousarray(v, dtype=np.float32) for k, v in out.items()}


# --------------------------------------------------------------------------
# device kernel
# --------------------------------------------------------------------------
def build_nc(n_samples=SPB):
    rows = n_samples * T
    nc = bacc.Bacc("TRN2", target_bir_lowering=False, debug=False)

    def mm(out, lhsT, rhs, **kw):
        nc.tensor.matmul(out, lhsT, rhs, **kw)

    xt_d = nc.dram_tensor("xt", [IN_SIZE, rows], FP16, kind="ExternalInput")
    wgi_d = nc.dram_tensor("wgi", [128, 2, 6 * PG], FP16, kind="ExternalInput")
    wtail_d = nc.dram_tensor("wtail", [128, 6 * PG], FP16, kind="ExternalInput")
    wgh_d = nc.dram_tensor("wgh", [PG, 3 * PG], FP16, kind="ExternalInput")
    wdiag_d = nc.dram_tensor("wdiag", [PG, PG], FP16, kind="ExternalInput")
    wkan_d = nc.dram_tensor("wkan", [128, 8, 257], FP16, kind="ExternalInput")
    bgru_d = nc.dram_tensor("bgru", [PG, 8], F32, kind="ExternalInput")
    ones_d = nc.dram_tensor("ones1", [1, NT2], FP16, kind="ExternalInput")
    yt_d = nc.dram_tensor("yt", [n_samples, 128, 8, 257], BF16, kind="ExternalOutput")

    with tile.TileContext(nc) as tc:
        with (
            tc.tile_pool(name="wts", bufs=1) as wp,
            tc.tile_pool(name="xin", bufs=3) as xp,
            tc.tile_pool(name="work", bufs=1) as kp,
            tc.tile_pool(name="outp", bufs=2) as op_,
            tc.tile_pool(name="psg", bufs=1, space="PSUM") as psg,
            tc.tile_pool(name="psk", bufs=1, space="PSUM") as psk,
        ):
            # ---- resident weights
            wgi_sb = wp.tile([128, 2, 6 * PG], FP16, tag="wgi")
            nc.sync.dma_start(wgi_sb[:], wgi_d[:])
            wtail_sb = wp.tile([128, 6 * PG], FP16, tag="wtail")
            nc.sync.dma_start(wtail_sb[:], wtail_d[:])
            wgh_sb = wp.tile([PG, 3 * PG], FP16, tag="wgh")
            nc.sync.dma_start(wgh_sb[:], wgh_d[:])
            wdiag_sb = wp.tile([PG, PG], FP16, tag="wdiag")
            nc.sync.dma_start(wdiag_sb[:], wdiag_d[:])
            wkan_sb = wp.tile([128, 8, 257], FP16, tag="wkan")
            nc.sync.dma_start(wkan_sb[:], wkan_d[:])
            bg = wp.tile([PG, 8], F32, tag="bgru")
            nc.sync.dma_start(bg[:], bgru_d[:])

            # persistent feat tiles: gap 40:64 and pad 105:128 stay zero,
            # partition 104 stays 1.0 (the KAN constant-term row).
            featb = []
            for i in range(2):
                f = wp.tile([128, NT2], FP16, tag=f"featP{i}", name=f"featP{i}")
                nc.vector.memset(f[32:64, :], 0.0)
                nc.vector.memset(f[96:128, :], 0.0)
                nc.sync.dma_start(f[104:105, :], ones_d[:])
                featb.append(f)

            S = [dict() for _ in range(n_samples)]

            def load_x(smp):
                st = S[smp]
                s0 = smp * T
                xm = xp.tile([128, 2, NT2], FP16, tag="xm", name="xm")
                for ci in range(2):
                    nc.sync.dma_start(xm[:, ci, :],
                                      xt_d[ci * 128:(ci + 1) * 128, s0:s0 + NT2])
                xt4 = xp.tile([128, NT2], FP16, tag="xt4", name="xt4")
                for q in range(4):
                    nc.sync.dma_start(xt4[32 * q:32 * q + 1, :],
                                      xt_d[256:257, s0:s0 + NT2])
                st["xm"], st["xt4"] = xm, xt4

            def gi_main(ps, xm, lyr, g, hs):
                c0 = (lyr * 3 + g) * PG
                for ci in range(2):
                    mm(ps[:], wgi_sb[:, ci, c0:c0 + PG], xm[:, ci, hs],
                       start=(ci == 0), stop=False)

            def gi_tail(ps, xt4, lyr, g, hs, q, stop=True):
                c0 = (lyr * 3 + g) * PG
                mm(ps[:], wtail_sb[32 * q:32 * q + 1, c0:c0 + PG],
                   xt4[32 * q:32 * q + 1, hs], start=False, stop=stop,
                   tile_position=(32 * q, 0))

            def emit_l0(smp):
                st = S[smp]
                xm, xt4 = st["xm"], st["xt4"]
                rt = kp.tile([PG, NT2], FP16, tag="rt", bufs=2)
                zt = kp.tile([PG, NT2], FP16, tag="zt", bufs=2)
                n1 = kp.tile([PG, NT2], FP16, tag="n1", bufs=2)
                h1 = kp.tile([PG, NT2], FP16, tag="h1", bufs=2)
                for h in range(2):
                    hs = slice(h * NT, (h + 1) * NT)
                    ps_r = psg.tile([PG, NT], F32, tag="G", name="ps_r", bufs=6)
                    gi_main(ps_r, xm, 0, 0, hs)
                    gi_tail(ps_r, xt4, 0, 0, hs, 0)
                    ps_z = psg.tile([PG, NT], F32, tag="G", name="ps_z", bufs=6)
                    gi_main(ps_z, xm, 0, 1, hs)
                    ps_n = psg.tile([PG, NT], F32, tag="G", name="ps_n", bufs=6)
                    gi_main(ps_n, xm, 0, 2, hs)
                    gi_tail(ps_z, xt4, 0, 1, hs, 1)
                    gi_tail(ps_n, xt4, 0, 2, hs, 2, stop=False)
                    nc.scalar.activation(rt[:, hs], ps_r[:], AF.Sigmoid, bias=bg[:, 0:1])
                    nc.scalar.activation(zt[:, hs], ps_z[:], AF.Sigmoid,
                                         bias=bg[:, 1:2], scale=-1.0)
                    # n-psum += diag(bhh_n1) @ r  (replaces a DVE stt)
                    mm(ps_n[:], wdiag_sb[:], rt[:, hs], start=False, stop=True)
                    nc.scalar.activation(n1[:, hs], ps_n[:], AF.Tanh, bias=bg[:, 3:4])
                    nc.vector.tensor_mul(h1[:, hs], zt[:, hs], n1[:, hs])  # (1-z1)*n1
                st["h1"] = h1

            def emit_l1(smp):
                st = S[smp]
                xm, xt4, h1 = st["xm"], st["xt4"], st["h1"]
                r2t = kp.tile([PG, NT2], FP16, tag="r2t", bufs=2)
                z2t = kp.tile([PG, NT2], FP16, tag="z2t", bufs=2)
                t2t = kp.tile([PG, NT2], FP16, tag="t2t", bufs=2)
                vt = kp.tile([PG, NT2], FP16, tag="vt", bufs=2)
                for h in range(2):
                    hs = slice(h * NT, (h + 1) * NT)
                    ps_r2 = psg.tile([PG, NT], F32, tag="G", name="ps_r2", bufs=6)
                    gi_main(ps_r2, xm, 1, 0, hs)
                    mm(ps_r2[:], wgh_sb[:, 0:PG], h1[:, hs], start=False, stop=False)
                    gi_tail(ps_r2, xt4, 1, 0, hs, 0)
                    ps_z2 = psg.tile([PG, NT], F32, tag="G", name="ps_z2", bufs=6)
                    gi_main(ps_z2, xm, 1, 1, hs)
                    mm(ps_z2[:], wgh_sb[:, PG:2 * PG], h1[:, hs], start=False, stop=False)
                    ps_n2 = psg.tile([PG, NT], F32, tag="G", name="ps_n2", bufs=6)
                    gi_main(ps_n2, xm, 1, 2, hs)
                    ps_p3 = psg.tile([PG, NT], F32, tag="G", name="ps_p3", bufs=6)
                    mm(ps_p3[:], wgh_sb[:, 2 * PG:3 * PG], h1[:, hs], start=True, stop=True)
                    gi_tail(ps_z2, xt4, 1, 1, hs, 1)
                    gi_tail(ps_n2, xt4, 1, 2, hs, 2)
                    nc.scalar.activation(r2t[:, hs], ps_r2[:], AF.Sigmoid, bias=bg[:, 4:5])
                    nc.scalar.activation(z2t[:, hs], ps_z2[:], AF.Sigmoid,
                                         bias=bg[:, 5:6], scale=-1.0)
                    nc.vector.scalar_tensor_tensor(
                        t2t[:, hs], ps_p3[:], bg[:, 6:7], r2t[:, hs],
                        op0=ALU.add, op1=ALU.mult)
                    nc.vector.tensor_add(vt[:, hs], t2t[:, hs], ps_n2[:])
                n2 = kp.tile([PG, NT2], FP16, tag="n2", bufs=2)
                nc.scalar.activation(n2[:], vt[:], AF.Tanh, bias=bg[:, 7:8])
                # feat = (1-z2)*n2 + z2*h1 = z2t'*(n2-h1) + h1   (z2t' = 1-z2)
                wdt = kp.tile([PG, NT2], FP16, tag="wdt", bufs=2)
                nc.vector.tensor_sub(wdt[:], n2[:], h1[:])
                ut = kp.tile([PG, NT2], FP16, tag="ut", bufs=2)
                nc.vector.tensor_mul(ut[:], z2t[:], wdt[:])
                feat = featb[smp % 2]
                nc.vector.tensor_add(feat[0:40, :], ut[0:40, :], h1[0:40, :])
                nc.vector.tensor_add(feat[BO:BO + 40, :], ut[BO:BO + 40, ::-1],
                                     h1[BO:BO + 40, ::-1])
                st["feat"] = feat
                # KAN power basis (all fp16); breadth-first so no DVE op waits
                # on the instruction right before it.
                s2 = kp.tile([128, NT2], FP16, tag="s2", bufs=2)
                nc.gpsimd.tensor_mul(s2[:], feat[:], feat[:])
                sg = kp.tile([128, NT2], FP16, tag="sg", bufs=2)
                nc.scalar.activation(sg[:], feat[:], AF.Sigmoid)
                rjs = []
                for ji, (tj, side) in enumerate(KNOTS):
                    rj = kp.tile([128, NT2], FP16, tag=f"rj{ji}", name=f"rj{ji}", bufs=2)
                    nc.vector.tensor_scalar(
                        rj[:], feat[:], float(tj), 0.0, op0=ALU.subtract,
                        op1=(ALU.min if side == "L" else ALU.max))
                    rjs.append(rj)
                rjc = kp.tile([128, NT2], FP16, tag="rjc", bufs=2)
                nc.vector.tensor_copy(rjc[:], rjs[1][:])
                rjc3 = kp.tile([128, NT2], FP16, tag="rjc3", bufs=2)
                nc.vector.tensor_copy(rjc3[:], rjs[3][:])
                sl = kp.tile([128, NT2], FP16, tag="sl", bufs=2)
                nc.gpsimd.tensor_mul(sl[:], sg[:], feat[:])
                s3 = kp.tile([128, NT2], FP16, tag="s3", bufs=2)
                nc.vector.tensor_mul(s3[:], s2[:], feat[:])
                qjs = []
                for ji in range(4):
                    qj = kp.tile([128, NT2], FP16, tag=f"qj{ji}", name=f"qj{ji}", bufs=2)
                    if ji % 2 == 0:
                        nc.gpsimd.tensor_mul(qj[:], rjs[ji][:], rjs[ji][:])
                    else:
                        nc.vector.tensor_mul(qj[:], rjs[ji][:],
                                             (rjc if ji == 1 else rjc3)[:])
                    qjs.append(qj)
                powers = [sl, feat, s2, s3]
                for ji in range(4):
                    pj = kp.tile([128, NT2], FP16, tag=f"pj{ji}", name=f"pj{ji}", bufs=2)
                    nc.vector.tensor_mul(pj[:], qjs[ji][:], rjs[ji][:])
                    powers.append(pj)
                st["powers"] = powers

            def emit_kan(smp):
                st = S[smp]
                powers = st["powers"]
                ob = op_.tile([128, 8, 257], BF16, tag="ob", name="ob")
                for c in range(8):
                    r0 = c * 128
                    msz = min(128, T - r0)
                    rs = slice(r0, r0 + msz)
                    po = psk.tile([128, 257], F32, tag="K", name="po", bufs=2)
                    for idx, p in enumerate(powers):
                        mm(po[0:msz, :], p[:, rs], wkan_sb[:, idx, :],
                           start=(idx == 0), stop=(idx == 7))
                    nc.scalar.activation(ob[0:msz, c, :], po[0:msz, :], AF.Sigmoid)
                nc.sync.dma_start(yt_d[smp, :, 0:7, :], ob[:, 0:7, :])
                nc.sync.dma_start(yt_d[smp, 0:104, 7, :], ob[0:104, 7, :])

            load_x(0)
            if n_samples > 1:
                load_x(1)
            for k in range(n_samples + 2):
                if k + 2 < n_samples:
                    load_x(k + 2)
                if k < n_samples:
                    emit_l0(k)
                if 0 <= k - 1 < n_samples:
                    emit_l1(k - 1)
                if 0 <= k - 2 < n_samples:
                    emit_kan(k - 2)
                    S[k - 2].clear()
    nc.compile()
    return nc


# --------------------------------------------------------------------------
# host entry point
# --------------------------------------------------------------------------
_NC_CACHE = {}


def _get_nc(n_samples=SPB, mode=None):
    key = n_samples
    if key not in _NC_CACHE:
        _NC_CACHE[key] = build_nc(n_samples)
    return _NC_CACHE[key]


def make_in_maps(inputs, n_samples=SPB, n_cores=N_CORES, mode=None):
    x = np.asarray(inputs["x"], dtype=np.float32)
    Wf = fold_weights(inputs)
    for k in ("wgi", "wtail", "wgh", "wdiag", "wkan", "ones1"):
        Wf[k] = np.ascontiguousarray(Wf[k].astype(np.float16))
    in_maps = []
    for c in range(n_cores):
        xc = x[c * n_samples:(c + 1) * n_samples].reshape(n_samples * T, IN_SIZE)
        xt = np.ascontiguousarray(xc.T.astype(np.float16))
        in_maps.append({"xt": xt, **Wf})
    return in_maps


def kernel(**inputs):
    x = np.asarray(inputs["x"], dtype=np.float32)
    assert x.shape == (B, T, IN_SIZE), x.shape
    nc = _get_nc(SPB)
    in_maps = make_in_maps(inputs)
    res = run_bass_kernel_spmd(nc, in_maps, list(range(N_CORES)))
    out = np.empty((B, T, OUT_SIZE), dtype=np.float32)
    for c in range(N_CORES):
        yt = np.asarray(res.results[c]["yt"], dtype=np.float32)  # (SPB,128,8,257)
        for s in range(SPB):
            y = yt[s].transpose(1, 0, 2).reshape(1024, OUT_SIZE)[:T]
            out[c * SPB + s] = 1.2 * y
    return out


if __name__ == "__main__":
    rng = np.random.default_rng(0)
    demo = {
        "x": rng.standard_normal((B, T, IN_SIZE), dtype=np.float32),
        "Wih_f": rng.standard_normal((2, 120, 257), dtype=np.float32) * 0.1,
        "Whh_f": rng.standard_normal((2, 120, 40), dtype=np.float32) * 0.1,
        "bih_f": rng.standard_normal((2, 120), dtype=np.float32) * 0.1,
        "bhh_f": rng.standard_normal((2, 120), dtype=np.float32) * 0.1,
        "Wih_b": rng.standard_normal((2, 120, 257), dtype=np.float32) * 0.1,
        "Whh_b": rng.standard_normal((2, 120, 40), dtype=np.float32) * 0.1,
        "bih_b": rng.standard_normal((2, 120), dtype=np.float32) * 0.1,
        "bhh_b": rng.standard_normal((2, 120), dtype=np.float32) * 0.1,
        "base_weight": rng.standard_normal((257, 80), dtype=np.float32) * 0.1,
        "spline_weight": rng.standard_normal((257, 80, 8), dtype=np.float32) * 0.1,
        "spline_scaler": np.ones((257, 80), dtype=np.float32),
        "slope": np.ones((257,), dtype=np.float32),
        "lengths": np.full((64,), 1000, dtype=np.int32),
    }
    out = kernel(**demo)
    print("kernel ran, out:", out.shape, out.dtype, float(out.min()), float(out.max()))
